# revision 57
# baseline (speedup 1.0000x reference)
"""Trainium2 Bass kernel for causal multi-head attention (B=4, T=2048, D=1024, H=16).

Sharding: 8 cores = 4 batches x 2 head-groups (8 heads each).
Per core pipeline (Tile framework, single SPMD program):
  phase 1(j): Q/K projections into transposed per-head-pair layout QT/KT [128, T],
           V projection into [t, 8*65] layout (65th col per head = ones -> rowsums).
           Emitted one q-range AHEAD (phase1(j+1) before norm/phase3(j)) so the PE
           instruction queue never head-of-line blocks at a j boundary; psum->sbuf
           copies run on the Scalar engine to keep DVE free for normalization.
  phase 2(j): per (q-range of 512, head-pair): causal attention in transposed
           layout: ST[k,q] = KT^T @ QT (row-packed matmul pair), PT = exp(ST) (ACT),
           mask only the 128-wide diagonal strip (one fused DVE mul over both heads),
           OT[hd+1,q] += [V|1]^T @ PT.  Rowsums staged at 32p-aligned partitions,
           one full-tile DVE reciprocal per j, broadcast via rank-1 PE matmul
           (ones[1,64].T @ row) into freed psum, normalize with DVE muls.
  phase 3(j): output projection + bias, bf16 ReduceScatter in two 512-row chunks
           per q-range (overlapped with later compute), direct DMA of each half.
Host: transpose/pack weights (single-DMA layouts), assemble [B, T, D] from
per-core [512, T] halves with the chunked-RS dout interleave.
"""

import numpy as np

B, T, D, H, HD = 4, 2048, 1024, 16, 64
NCORES = 8
NP = 4          # head pairs per core
NJ = 4          # q-ranges of 512
QW = 512
TB = T // 128   # 16

_CACHE = {}
import os
GP_MASK = os.environ.get("KV_GPMASK", "0") == "1"    # masks on gpsimd vs vector
LN_NORM = os.environ.get("KV_LNNORM", "0") == "1"    # exp(-ln) norm vs vector.reciprocal
RS_BF16 = os.environ.get("KV_RSBF16", "1") == "1"    # bf16 chunked RS vs fp32 single RS


def _build_nc():
    import concourse.mybir as mybir
    import concourse.tile as tile
    from concourse import bacc

    F32 = mybir.dt.float32
    F32R = mybir.dt.float32r
    BF16 = mybir.dt.bfloat16
    F16 = mybir.dt.float16
    FP8 = mybir.dt.float8e4
    DR = mybir.MatmulPerfMode.DoubleRow
    AF = mybir.ActivationFunctionType
    ALU = mybir.AluOpType

    nc = bacc.Bacc(None, target_bir_lowering=False)
    xt_d = nc.declare_dram_parameter("xt", [D, T], BF16, isOutput=False)
    wq_d = nc.declare_dram_parameter("wq", [128, 8 * 512], BF16, isOutput=False)
    wk_d = nc.declare_dram_parameter("wk", [128, 8 * 512], BF16, isOutput=False)
    wv_d = nc.declare_dram_parameter("wv", [128, 8 * 512], BF16, isOutput=False)
    wo_d = nc.declare_dram_parameter("wo", [128, 4 * D], BF16, isOutput=False)
    bias_d = nc.declare_dram_parameter("bias", [128, 8], F32, isOutput=False)
    mask_d = nc.declare_dram_parameter("mask", [128, 256], BF16, isOutput=False)
    YDT = BF16 if RS_BF16 else F32
    if RS_BF16:
        # [j, half, 256 douts, q] — contiguous per RS chunk so the collective
        # can scatter straight into the output tensor
        yt_d = nc.declare_dram_parameter("yt", [NJ, 2, 256, QW], YDT, isOutput=True)
    else:
        yt_d = nc.declare_dram_parameter("yt", [512, T], YDT, isOutput=True)

    RG = [[0, 1], [2, 3], [4, 5], [6, 7]]

    with tile.TileContext(nc) as tc:
        with (
            tc.tile_pool(name="persist", bufs=1) as pers,
            tc.tile_pool(name="work", bufs=1) as work,
            tc.tile_pool(name="dram", bufs=1, space="DRAM") as dram,
            tc.tile_pool(name="psum", bufs=1, space="PSUM") as psum,
        ):
            qt = pers.tile([128, NP, T], BF16)
            kt = pers.tile([128, NP, T], BF16)
            v = pers.tile([128, TB, 8 * 65], BF16)
            ot = pers.tile([128, NP, T], BF16)
            m0 = pers.tile([128, 2, 128], BF16)
            wo = pers.tile([128, 4, D], BF16)
            bias = pers.tile([128, 8], F32)
            wq = pers.tile([128, 8, 512], BF16)
            wk = pers.tile([128, 8, 512], BF16)
            wv = pers.tile([128, 8, 512], BF16)
            # per-j rowsum staging: head-pair p's rows live at partition 32p so every
            # AP starts at a {0,32,64,96} boundary (A tile: head 2p, B tile: head 2p+1)
            rsA = pers.tile([128, QW], BF16)
            rsB = pers.tile([128, QW], BF16)
            recbA = pers.tile([128, QW], BF16)
            recbB = pers.tile([128, QW], BF16)
            nc.gpsimd.memset(rsA[:], 1.0)
            nc.gpsimd.memset(rsB[:], 1.0)
            ones1 = pers.tile([128, 64], BF16)
            nc.gpsimd.memset(ones1[:], 1.0)
            nc.sync.dma_start(out=m0[:], in_=mask_d[:])
            for c in range(8):
                nc.sync.dma_start(out=wq[:, c, :], in_=wq_d[:, c * 512:(c + 1) * 512])

            yt_part = dram.tile([NJ, 1024, QW], YDT)
            yt_rs = dram.tile([NJ, 4, 128, QW], YDT)

            def phase1(j):
                """Projections for t-range j. Emitted BEFORE the previous range's
                normalization+phase3 so the PE queue never head-of-line blocks at
                a j boundary. psum->sbuf copies go to Scalar/GpSimd, keeping DVE
                free for the normalization chain."""
                xsb = work.tile([128, 8, QW], BF16, tag="xsb", bufs=3)
                for c in range(8):
                    nc.sync.dma_start(
                        out=xsb[:, c, :],
                        in_=xt_d[c * 128:(c + 1) * 128, j * QW:(j + 1) * QW],
                    )
                if j == 0:
                    # wk/wv stream in per-chunk AFTER j=0's x so the first Q matmuls
                    # aren't queued behind 2MB of weights they don't need yet
                    for c in range(8):
                        nc.sync.dma_start(out=wk[:, c, :], in_=wk_d[:, c * 512:(c + 1) * 512])
                    for c in range(8):
                        nc.sync.dma_start(out=wv[:, c, :], in_=wv_d[:, c * 512:(c + 1) * 512])
                    nc.sync.dma_start(out=bias[:], in_=bias_d[:])
                    nc.sync.dma_start(out=wo[:], in_=wo_d[:])
                for p in range(NP):
                    for w_sb, dst in ((wq, qt), (wk, kt)):
                        acc = psum.tile([128, QW], F32, tag="small", bufs=2)
                        for c in range(8):
                            nc.tensor.matmul(
                                acc[:],
                                w_sb[:, c, p * 128:(p + 1) * 128],
                                xsb[:, c, :],
                                start=(c == 0),
                                stop=(c == 7),
                            )
                        nc.scalar.copy(dst[:, p, j * QW:(j + 1) * QW], acc[:])
                for sub in range(4):
                    i = 4 * j + sub
                    acc = psum.tile([128, QW], F32, tag="small", bufs=2)
                    for c in range(8):
                        nc.tensor.matmul(
                            acc[:],
                            xsb[:, c, sub * 128:(sub + 1) * 128],
                            wv[:, c, :],
                            start=(c == 0),
                            stop=(c == 7),
                        )
                    vblk = v[:, i, :].rearrange("p (h c) -> p h c", c=65)
                    nc.scalar.copy(
                        vblk[:, :, 0:64],
                        acc[:].rearrange("p (h c) -> p h c", c=64),
                    )
                    nc.gpsimd.memset(vblk[:, :, 64:65], 1.0)

            phase1(0)
            pending = []
            for j in range(NJ):
                # ---------------- phase 2(j): attention ----------------
                ocps = []
                for p in range(NP):
                    if p == 1 and pending:
                        pending.pop(0)()
                    hA, hB = 2 * p, 2 * p + 1
                    o_A = psum.tile([65, QW], F32, tag="o", bufs=2)
                    o_B = psum.tile([65, QW], F32, tag="o", bufs=2)
                    nkb = 4 * j + 4
                    for kb in range(nkb):
                        o = kb - 4 * j  # diagonal offset; < 0 means full block
                        lo = 128 * o if o > 0 else 0  # first live q col in range
                        W = QW - lo
                        st = psum.tile([128, 1024], F32, tag="st", bufs=2)
                        kcols = slice(kb * 128, (kb + 1) * 128)
                        qcols = slice(j * QW + lo, (j + 1) * QW)
                        nc.tensor.matmul(
                            st[:, lo:QW],
                            kt[0:64, p, kcols],
                            qt[0:64, p, qcols],
                            start=True, stop=True, tile_position=(0, 0),
                        )
                        nc.tensor.matmul(
                            st[:, QW + lo:2 * QW],
                            kt[64:128, p, kcols],
                            qt[64:128, p, qcols],
                            start=True, stop=True, tile_position=(64, 0),
                        )
                        pt = work.tile([128, 1024], BF16, tag="pt", bufs=6)
                        nc.scalar.activation(
                            pt[:].rearrange("p (h q) -> p h q", h=2)[:, :, lo:QW],
                            st[:].rearrange("p (h q) -> p h q", h=2)[:, :, lo:QW],
                            AF.Exp,
                        )
                        if o >= 0:
                            # only the 128-wide diagonal strip needs masking
                            eng = nc.gpsimd if GP_MASK else nc.vector
                            ptv = pt[:].rearrange("p (h q) -> p h q", h=2)[:, :, lo:lo + 128]
                            eng.tensor_mul(ptv, ptv, m0[:])
                        nc.tensor.matmul(
                            o_A[:, lo:QW],
                            v[:, kb, hA * 65:(hA + 1) * 65],
                            pt[:, lo:QW],
                            start=(kb == 0), stop=(kb == nkb - 1),
                        )
                        nc.tensor.matmul(
                            o_B[:, lo:QW],
                            v[:, kb, hB * 65:(hB + 1) * 65],
                            pt[:, QW + lo:2 * QW],
                            start=(kb == 0), stop=(kb == nkb - 1),
                        )
                    # copy psum accumulators out so the o slots free early;
                    # stash rowsums into the per-j batch tile for one ACT reciprocal
                    ocp = work.tile([64, 1024], F32, tag="ocp", bufs=5)
                    nc.vector.tensor_copy(ocp[:, 0:QW], o_A[0:64, :])
                    nc.vector.tensor_copy(ocp[:, QW:1024], o_B[0:64, :])
                    nc.vector.tensor_copy(rsA[32 * p:32 * p + 1, :], o_A[64:65, :])
                    nc.vector.tensor_copy(rsB[32 * p:32 * p + 1, :], o_B[64:65, :])
                    ocps.append(ocp)
                if j + 1 < NJ:
                    phase1(j + 1)

                if LN_NORM:
                    # 1/x = exp(-ln x): both fns live in one ACT table set (no switch)
                    nc.scalar.activation(recbA[:], rsA[:], AF.Ln)
                    nc.scalar.activation(recbA[:], recbA[:], AF.Exp, scale=-1.0)
                    nc.scalar.activation(recbB[:], rsB[:], AF.Ln)
                    nc.scalar.activation(recbB[:], recbB[:], AF.Exp, scale=-1.0)
                else:
                    with nc.allow_low_precision(reason="bf16 softmax recip is plenty"):
                        nc.vector.reciprocal(recbA[:], rsA[:])
                        nc.vector.reciprocal(recbB[:], rsB[:])

                jr = slice(j * QW, (j + 1) * QW)
                for p in range(NP):
                    ocp = ocps[p]
                    # broadcast recip rows across partitions with a rank-1 matmul
                    # (ones[1,64].T @ row[1,512]) into freed "o"-pool psum slots;
                    # matmul base partition must be 0/32/64, so stage p=3 at 0
                    if p == 3:
                        r3 = work.tile([1, 1024], BF16, tag="r3", bufs=2)
                        nc.vector.tensor_copy(r3[:, 0:QW], recbA[96:97, :])
                        nc.vector.tensor_copy(r3[:, QW:1024], recbB[96:97, :])
                        rowA, rowB, bp = r3[:, 0:QW], r3[:, QW:1024], 0
                    else:
                        rowA = recbA[32 * p:32 * p + 1, :]
                        rowB = recbB[32 * p:32 * p + 1, :]
                        bp = 32 * p
                    bcA = psum.tile([65, QW], F32, tag="o", bufs=2)
                    bcB = psum.tile([65, QW], F32, tag="o", bufs=2)
                    nc.tensor.matmul(
                        bcA[0:64, :], ones1[bp:bp + 1, :], rowA,
                        start=True, stop=True,
                    )
                    nc.tensor.matmul(
                        bcB[0:64, :], ones1[bp:bp + 1, :], rowB,
                        start=True, stop=True,
                    )
                    nc.vector.tensor_mul(ot[0:64, p, jr], ocp[0:64, 0:QW], bcA[0:64, :])
                    nc.vector.tensor_mul(ot[64:128, p, jr], ocp[0:64, QW:1024], bcB[0:64, :])

                # ---------------- phase 3(j): output projection + chunked RS ----------------
                def phase3(j=j):
                    jr = slice(j * QW, (j + 1) * QW)
                    for n in range(8):
                        yps = psum.tile([128, QW], F32, tag="st", bufs=2)
                        for c in range(4):
                            nc.tensor.matmul(
                                yps[:],
                                wo[:, c, n * 128:(n + 1) * 128],
                                ot[:, c, jr],
                                start=(c == 0), stop=(c == 3),
                            )
                        ysb = work.tile([128, QW], YDT, tag="ysb", bufs=3)
                        nc.vector.tensor_scalar_add(ysb[:], yps[:], bias[:, n:n + 1])
                        nc.sync.dma_start(
                            out=yt_part[j, n * 128:(n + 1) * 128, :], in_=ysb[:]
                        )
                        if RS_BF16 and n % 4 == 3:
                            h = n // 4
                            nc.gpsimd.collective_compute(
                                "ReduceScatter",
                                ALU.add,
                                replica_groups=RG,
                                ins=[yt_part[j, h * 512:(h + 1) * 512].opt()],
                                outs=[yt_rs[j, 2 * h:2 * h + 2]
                                      .rearrange("h p q -> (h p) q").opt()],
                            )
                            nc.sync.dma_start(
                                out=yt_d[j, h],
                                in_=yt_rs[j, 2 * h:2 * h + 2].rearrange("h p q -> (h p) q"),
                            )
                    if not RS_BF16:
                        nc.gpsimd.collective_compute(
                            "ReduceScatter",
                            ALU.add,
                            replica_groups=RG,
                            ins=[yt_part[j].opt()],
                            outs=[yt_rs[j].rearrange("h p q -> (h p) q").opt()],
                        )
                        nc.sync.dma_start(
                            out=yt_d[:, jr], in_=yt_rs[j].rearrange("h p q -> (h p) q")
                        )
                pending.append(phase3)
            while pending:
                pending.pop(0)()

    nc.finalize()
    return nc


def _prep_inputs(x, Wq, Wk, Wv, Wo, bo):
    """Build the 8 per-core input maps (host-side layout prep only)."""
    import ml_dtypes

    scale = 1.0 / np.sqrt(np.float32(HD))
    kr = np.arange(128, dtype=np.float32)[:, None]
    qc = np.arange(128, dtype=np.float32)[None, :]
    tri = (qc >= kr).astype(ml_dtypes.bfloat16)
    m0 = np.ascontiguousarray(np.concatenate([tri, tri], axis=1))

    def chunk(a):  # [D, N] -> [128, (D//128)*N] grouping rows by 128-chunks
        dd, n = a.shape
        return np.ascontiguousarray(
            a.reshape(dd // 128, 128, n).transpose(1, 0, 2).reshape(128, -1)
        )

    in_maps = []
    for c in range(NCORES):
        b, g = c // 2, c % 2
        hs = slice(g * 8, (g + 1) * 8)
        xt = np.ascontiguousarray(x[b].T).astype(ml_dtypes.bfloat16)
        wq = chunk(Wq[hs].reshape(512, D).T * scale).astype(ml_dtypes.bfloat16)
        wk = chunk(Wk[hs].reshape(512, D).T).astype(ml_dtypes.bfloat16)
        wv = chunk(Wv[hs].reshape(512, D).T).astype(ml_dtypes.bfloat16)
        wo = chunk(Wo[:, g * 512:(g + 1) * 512].T).astype(ml_dtypes.bfloat16)
        if g == 0:
            bias = np.ascontiguousarray(bo.reshape(8, 128).T)
        else:
            bias = np.zeros((128, 8), np.float32)
        in_maps.append(
            {"xt": xt, "wq": wq, "wk": wk, "wv": wv, "wo": wo, "bias": bias, "mask": m0}
        )
    return in_maps


def _run(inputs, trace=False, trace_cores=None):
    from concourse.bass_utils import run_bass_kernel_spmd

    if "nc" not in _CACHE:
        _CACHE["nc"] = _build_nc()
    nc = _CACHE["nc"]
    in_maps = _prep_inputs(
        inputs["x"], inputs["Wq"], inputs["Wk"], inputs["Wv"], inputs["Wo"], inputs["bo"]
    )
    r = run_bass_kernel_spmd(
        nc, in_maps, list(range(NCORES)), trace=trace, trace_cores=trace_cores
    )
    y = _gather([r.results[c]["yt"] for c in range(NCORES)])
    return y, r


def _gather(yts):
    """Assemble [B, T, D] from per-core yt [512, T] with chunked-RS dout layout:
    even core holds douts [0:256]+[512:768], odd holds [256:512]+[768:1024]."""
    y = np.empty((B, T, D), np.float32)
    for b in range(B):
        ev = np.asarray(yts[2 * b], dtype=np.float32)
        od = np.asarray(yts[2 * b + 1], dtype=np.float32)
        if RS_BF16:
            # yt comes back as [NJ, 2, 256, QW]: chunk (j, h) holds douts
            # h*512+[0:256] (even core) / h*512+[256:512] (odd core), cols j*QW..
            yt = np.empty((D, T), np.float32)
            for j in range(NJ):
                jr = slice(j * QW, (j + 1) * QW)
                for h in range(2):
                    yt[h * 512:h * 512 + 256, jr] = ev[j, h]
                    yt[h * 512 + 256:(h + 1) * 512, jr] = od[j, h]
        else:
            yt = np.concatenate([ev, od], axis=0)
        y[b] = yt.T
    return y


def kernel(**inputs):
    y, _ = _run(inputs, trace=False)
    return y

